# revision 1
# baseline (speedup 1.0000x reference)
"""Tricubic B-spline grid interpolation (CubicBSplineGrid3d) on 8 Trainium2 cores.

Strategy (data-parallel over queries, per sharding hint):
  * Host: pad grid (64,64,64,32) -> (67,67,67,32) edge-replicated, then pack the
    4x4 (d,h)-neighborhood redundantly in fp16:
        Q[d, h, w, c, i, j] = Gpad[d+i, h+j, w, c]   (fp16, ~281 MB)
    so each query's full 4x4x4x32 neighborhood is ONE 4KB contiguous run
    (4 consecutive 512-element rows), base=(sd*64+sh)*67+sw.
  * Device (per core, 16384 queries = 128 blocks of 128):
      stage 1: cubic basis weights + base indices; PE-transpose to
               query-on-partition layout; wdh products (fp16, b-major).
      main loop (software-pipelined by one compute group of 4 blocks):
        - one indirect DMA gathers 8 blocks at once ([128, 8, 2048] fp16)
        - pass A (contract w): per block, 4x tensor_scalar mults; k=0,1 on
          DVE (4x perf mode), k=2,3 on the Act engine; per-k partial
          products then tree-added (t01 on DVE, t23 on GpSimd/Pool, final
          on DVE), all batched over 4 blocks.
        - pass B (contract d,h): one fp16 tensor_tensor multiply against
          broadcast wd*wh weights (2x mode) + a 2x-mode add-tree over the
          16 (i,j) slots, final level writes f32.
        - outputs accumulate in SBUF, one 2KB/partition DMA per 16 blocks
          into a [query-in-block, block, c] DRAM layout.
  * Host: transpose per-core outputs to [block, query, c] order and concat.
"""
import sys

for _p in ("/opt/trn_rl_repo",):
    if _p not in sys.path:
        sys.path.insert(0, _p)

import numpy as np

N_CORES = 8
B_GLOBAL = 131072
B_LOCAL = B_GLOBAL // N_CORES          # 16384
NBLK = B_LOCAL // 128                  # 128 blocks of 128 queries
GD = GH = GW = 64                      # grid spatial dims
GC = 32                                # channels
QROWS = GD * GH * (GW + 3)             # 64*64*67 = 274432
QROWLEN = 4 * 4 * GC                   # 512 elements per (d,h,w) row
GLEN = 4 * QROWLEN                     # 2048 elements gathered per query
NGATH = 8                              # blocks per indirect gather
NCOMP = 4                              # blocks per compute group
NOUT = 16                              # blocks per output DMA

_nc_cache = None


def _build_nc():
    """Build + compile the per-core Bass program (identical on all cores)."""
    from concourse import bacc, mybir
    from concourse.bass import IndirectOffsetOnAxis
    from concourse.tile import TileContext
    from concourse.masks import make_identity

    f32, f16, i32 = mybir.dt.float32, mybir.dt.float16, mybir.dt.int32
    Alu = mybir.AluOpType
    Act = mybir.ActivationFunctionType
    P = 128

    nc = bacc.Bacc("TRN2", target_bir_lowering=False, debug=False,
                   num_devices=N_CORES)
    u_t = nc.dram_tensor("u", [B_LOCAL, 3], f32, kind="ExternalInput")
    q_t = nc.dram_tensor("q", [QROWS, QROWLEN], f16, kind="ExternalInput")
    o_t = nc.dram_tensor("o", [P, NBLK * GC], f32, kind="ExternalOutput")

    with TileContext(nc) as tc:
        with (
            tc.tile_pool(name="persist", bufs=1) as pp,
            tc.tile_pool(name="stage1", bufs=1) as s1,
            tc.tile_pool(name="psum", bufs=2, space="PSUM") as psum,
            tc.tile_pool(name="g", bufs=2) as gp,
            tc.tile_pool(name="t4", bufs=2) as tp,
            tc.tile_pool(name="ta", bufs=2) as ta,
            tc.tile_pool(name="tb", bufs=2) as tb,
            tc.tile_pool(name="a4", bufs=2) as ap_,
            tc.tile_pool(name="p4", bufs=2) as p4p,
            tc.tile_pool(name="rt", bufs=2) as rt,
            tc.tile_pool(name="o", bufs=2) as op_,
        ):
            # ---------- stage 1: weights + indices (block layout) ----------
            # U[p, n, a] = u[p*128 + n, a]; per-partition 1536B contiguous.
            U = s1.tile([P, 384], f32)
            nc.sync.dma_start(
                out=U[:, :], in_=u_t[:, :].rearrange("(p n) c -> p (n c)", p=P))
            X = s1.tile([P, 384], f32)
            nc.vector.tensor_scalar(X[:, :], U[:, :], float(GD - 1), None, Alu.mult)
            # floor via round-to-nearest cast + correction
            Si = s1.tile([P, 384], i32)
            nc.vector.tensor_copy(out=Si[:, :], in_=X[:, :])
            Sf = s1.tile([P, 384], f32)
            nc.vector.tensor_copy(out=Sf[:, :], in_=Si[:, :])
            D = s1.tile([P, 384], f32)
            nc.vector.tensor_tensor(out=D[:, :], in0=X[:, :], in1=Sf[:, :],
                                    op=Alu.subtract)
            M = s1.tile([P, 384], f32)
            nc.vector.tensor_scalar(M[:, :], D[:, :], 0.0, None, Alu.is_lt)
            S = s1.tile([P, 384], f32)
            nc.vector.tensor_tensor(out=S[:, :], in0=Sf[:, :], in1=M[:, :],
                                    op=Alu.subtract)
            T = s1.tile([P, 384], f32)
            nc.vector.tensor_tensor(out=T[:, :], in0=X[:, :], in1=S[:, :],
                                    op=Alu.subtract)

            S3 = S[:, :].rearrange("p (n c) -> p n c", c=3)
            # base = (sd*64 + sh)*67 + sw
            Bse = s1.tile([P, 128], f32)
            nc.vector.scalar_tensor_tensor(
                out=Bse[:, :], in0=S3[:, :, 0], scalar=float(GH),
                in1=S3[:, :, 1], op0=Alu.mult, op1=Alu.add)
            nc.vector.scalar_tensor_tensor(
                out=Bse[:, :], in0=Bse[:, :], scalar=float(GW + 3),
                in1=S3[:, :, 2], op0=Alu.mult, op1=Alu.add)

            # cubic basis weights on [128, 384] (all 3 axes at once)
            T2 = s1.tile([P, 384], f32)
            nc.vector.tensor_tensor(out=T2[:, :], in0=T[:, :], in1=T[:, :],
                                    op=Alu.mult)
            T3 = s1.tile([P, 384], f32)
            nc.vector.tensor_tensor(out=T3[:, :], in0=T2[:, :], in1=T[:, :],
                                    op=Alu.mult)
            sixth = 1.0 / 6.0
            W0 = s1.tile([P, 384], f32)
            nc.vector.tensor_scalar(W0[:, :], T3[:, :], -sixth, None, Alu.mult)
            nc.vector.scalar_tensor_tensor(out=W0[:, :], in0=T2[:, :], scalar=0.5,
                                           in1=W0[:, :], op0=Alu.mult, op1=Alu.add)
            nc.vector.scalar_tensor_tensor(out=W0[:, :], in0=T[:, :], scalar=-0.5,
                                           in1=W0[:, :], op0=Alu.mult, op1=Alu.add)
            nc.vector.tensor_scalar(W0[:, :], W0[:, :], sixth, None, Alu.add)
            W1 = s1.tile([P, 384], f32)
            nc.vector.tensor_scalar(W1[:, :], T3[:, :], 0.5, None, Alu.mult)
            nc.vector.scalar_tensor_tensor(out=W1[:, :], in0=T2[:, :], scalar=-1.0,
                                           in1=W1[:, :], op0=Alu.mult, op1=Alu.add)
            nc.vector.tensor_scalar(W1[:, :], W1[:, :], 2.0 / 3.0, None, Alu.add)
            W3 = s1.tile([P, 384], f32)
            nc.vector.tensor_scalar(W3[:, :], T3[:, :], sixth, None, Alu.mult)
            # w2 = 1 - w0 - w1 - w3  (partition of unity)
            W2 = s1.tile([P, 384], f32)
            nc.vector.tensor_tensor(out=W2[:, :], in0=W0[:, :], in1=W1[:, :],
                                    op=Alu.add)
            nc.vector.tensor_tensor(out=W2[:, :], in0=W2[:, :], in1=W3[:, :],
                                    op=Alu.add)
            nc.vector.tensor_scalar(W2[:, :], W2[:, :], -1.0, 1.0,
                                    Alu.mult, Alu.add)

            # ---------- transposes to query-on-partition layout ----------
            ident = pp.tile([P, P], f32)
            make_identity(nc, ident[:, :])

            TD = pp.tile([P, 512], f32)   # wd_i  at cols i*128 + b
            TH = pp.tile([P, 512], f32)   # wh_j  at cols j*128 + b
            TW = pp.tile([P, 512], f32)   # ww_k  at cols k*128 + b
            FB = pp.tile([P, 128], f32)   # base  [query, block]
            Ws = [W0, W1, W2, W3]

            def transpose_into(dst_ap, src_ap):
                pt = psum.tile([P, P], f32, space="PSUM")
                nc.tensor.transpose(out=pt[:, :], in_=src_ap, identity=ident[:, :])
                nc.vector.tensor_copy(out=dst_ap, in_=pt[:, :])

            for a, Tt in ((0, TD), (1, TH), (2, TW)):
                for i in range(4):
                    w3v = Ws[i][:, :].rearrange("p (n c) -> p n c", c=3)
                    transpose_into(Tt[:, i * 128:(i + 1) * 128], w3v[:, :, a])
            transpose_into(FB[:, :], Bse[:, :])

            IdxI = pp.tile([P, 128], i32)
            nc.vector.tensor_copy(out=IdxI[:, :], in_=FB[:, :])

            # WDHt[q, b*16 + (i*4+j)] = wd_i[q,b] * wh_j[q,b]   (fp16, b-major)
            WDHt = pp.tile([P, NBLK * 16], f16)
            wv = WDHt[:, :].rearrange("p (b ij) -> p b ij", ij=16)
            for i in range(4):
                for j in range(4):
                    nc.vector.tensor_tensor(
                        out=wv[:, :, i * 4 + j],
                        in0=TD[:, i * 128:(i + 1) * 128],
                        in1=TH[:, j * 128:(j + 1) * 128],
                        op=Alu.mult)

            # ---------- main loop (pipelined by one compute group) ----------
            NG = NBLK // NGATH            # 16 gather groups
            NCG = NBLK // NCOMP           # 32 compute groups
            state = {}                    # per-group tiles for deferred pass B

            def emit_pass_a(n):
                """Pass A for compute group n: 16 tensor_scalar mults."""
                T4 = tp.tile([P, NCOMP, 4, QROWLEN], f16)
                b0 = n * NCOMP
                for blk in range(NCOMP):
                    b = b0 + blk
                    G = state.pop(("G", b))
                    for k in (0, 1):      # DVE, 4x perf mode
                        nc.vector.tensor_scalar(
                            T4[:, blk, k, :],
                            G[:, k * QROWLEN:(k + 1) * QROWLEN],
                            TW[:, k * 128 + b:k * 128 + b + 1], None, Alu.mult)
                    for k in (2, 3):      # Act engine
                        nc.scalar.activation(
                            T4[:, blk, k, :],
                            G[:, k * QROWLEN:(k + 1) * QROWLEN],
                            Act.Copy, bias=0.0,
                            scale=TW[:, k * 128 + b:k * 128 + b + 1])
                state[("T", n)] = T4

            def emit_pass_b(n):
                """Pass B for compute group n: w-sum tree, dh-mult, ij-tree."""
                T4 = state.pop(("T", n))
                b0 = n * NCOMP
                t01 = ta.tile([P, NCOMP, QROWLEN], f16)
                nc.vector.tensor_tensor(out=t01[:, :, :], in0=T4[:, :, 0, :],
                                        in1=T4[:, :, 1, :], op=Alu.add)
                t23 = tb.tile([P, NCOMP, QROWLEN], f16)
                nc.vector.tensor_tensor(out=t23[:, :, :], in0=T4[:, :, 2, :],
                                        in1=T4[:, :, 3, :], op=Alu.add)
                A4 = ap_.tile([P, NCOMP, QROWLEN], f16)
                nc.vector.tensor_tensor(out=A4[:, :, :], in0=t01[:, :, :],
                                        in1=t23[:, :, :], op=Alu.add)
                # multiply by wd*wh, broadcast over channels
                A4v = A4[:, :, :].rearrange("p blk (c ij) -> p blk c ij", ij=16)
                wb = (WDHt[:, b0 * 16:(b0 + NCOMP) * 16]
                      .rearrange("p (blk ij) -> p blk ij", ij=16)
                      .rearrange("p blk (x ij) -> p blk x ij", x=1)
                      .to_broadcast([P, NCOMP, GC, 16]))
                P4 = p4p.tile([P, NCOMP, GC, 16], f16)
                nc.vector.tensor_tensor(out=P4[:, :, :, :], in0=A4v[:, :, :, :],
                                        in1=wb, op=Alu.mult)
                # add-tree over the 16 (i,j) slots
                R8 = rt.tile([P, NCOMP, GC, 8], f16)
                nc.gpsimd.tensor_tensor(out=R8[:, :, :, :], in0=P4[:, :, :, 0:8],
                                        in1=P4[:, :, :, 8:16], op=Alu.add)
                R4 = rt.tile([P, NCOMP, GC, 4], f16)
                nc.vector.tensor_tensor(out=R4[:, :, :, :], in0=R8[:, :, :, 0:4],
                                        in1=R8[:, :, :, 4:8], op=Alu.add)
                R2 = rt.tile([P, NCOMP, GC, 2], f16)
                nc.vector.tensor_tensor(out=R2[:, :, :, :], in0=R4[:, :, :, 0:2],
                                        in1=R4[:, :, :, 2:4], op=Alu.add)
                if n % (NOUT // NCOMP) == 0:
                    state["O"] = op_.tile([P, NOUT // NCOMP, NCOMP, GC], f32,
                                          name=f"Oacc{n}")
                O = state["O"]
                nc.vector.tensor_tensor(
                    out=O[:, n % (NOUT // NCOMP), :, :], in0=R2[:, :, :, 0],
                    in1=R2[:, :, :, 1], op=Alu.add)
                if n % (NOUT // NCOMP) == (NOUT // NCOMP) - 1:
                    gg = n // (NOUT // NCOMP)
                    nc.sync.dma_start(
                        out=o_t[:, gg * NOUT * GC:(gg + 1) * NOUT * GC],
                        in_=O[:, :, :, :].rearrange("p a b c -> p (a b c)"))

            for n in range(NCG):
                for blk in range(NCOMP):
                    b = n * NCOMP + blk
                    G = gp.tile([P, GLEN], f16, name="G", bufs=12)
                    nc.gpsimd.indirect_dma_start(
                        out=G[:, :],
                        out_offset=None,
                        in_=q_t[:, :],
                        in_offset=IndirectOffsetOnAxis(
                            ap=IdxI[:, b:b + 1], axis=0),
                    )
                    state[("G", b)] = G
                emit_pass_a(n)
                if n >= 1:
                    emit_pass_b(n - 1)
            emit_pass_b(NCG - 1)
    nc.compile()
    return nc


def _pack_grid(grid: np.ndarray) -> np.ndarray:
    """(64,64,64,32) -> [QROWS, QROWLEN] fp16 with
    Q[d,h,w, c,i,j] = Gpad[d+i, h+j, w, c]."""
    gp = np.pad(grid, ((1, 2), (1, 2), (1, 2), (0, 0)), mode="edge")
    win = np.lib.stride_tricks.sliding_window_view(gp, (4, 4), axis=(0, 1))
    # win: [64, 64, 67, 32, 4, 4] = (d, h, w, c, i, j); ij innermost so the
    # on-device (d,h) contraction can tree-reduce contiguous slots.
    q = np.ascontiguousarray(win, dtype=np.float16)
    return q.reshape(QROWS, QROWLEN)


def kernel(u: np.ndarray, grid: np.ndarray) -> np.ndarray:
    global _nc_cache
    from concourse.bass_utils import run_bass_kernel_spmd

    assert u.shape == (B_GLOBAL, 3) and grid.shape == (GD, GH, GW, GC)
    if _nc_cache is None:
        _nc_cache = _build_nc()
    nc = _nc_cache

    q = _pack_grid(np.asarray(grid, dtype=np.float32))
    u = np.ascontiguousarray(u, dtype=np.float32)
    in_maps = [
        {"u": u[c * B_LOCAL:(c + 1) * B_LOCAL], "q": q} for c in range(N_CORES)
    ]
    res = run_bass_kernel_spmd(nc, in_maps, core_ids=list(range(N_CORES)))
    out = np.concatenate(
        [res.results[c]["o"].reshape(128, NBLK, GC).transpose(1, 0, 2)
         .reshape(B_LOCAL, GC) for c in range(N_CORES)], axis=0)
    return np.ascontiguousarray(out, dtype=np.float32)


if __name__ == "__main__":
    # quick self-run with random inputs
    rng = np.random.default_rng(0)
    grid = rng.standard_normal((GD, GH, GW, GC), dtype=np.float32)
    u = rng.random((B_GLOBAL, 3), dtype=np.float32)
    out = kernel(u, grid)
    print("out", out.shape, out.dtype, float(np.abs(out).mean()))

